# revision 7
# baseline (speedup 1.0000x reference)
"""Multi-head causal self-attention (B=4, T=1024, d_model=2048, 16 heads of 128)
for 8 Trainium2 NeuronCores.

Sharding: hybrid data x tensor parallel. Core c handles batch b = c//2 and
head group g = c%2 (8 heads per core). Each core computes q/k/v projections
for its 8 heads, causal flash-style attention, and the out-projection rows
for those heads, producing a partial [1024, 2048] output (fp16) for its
batch. The host sums the two partials per batch and adds the output bias.

Key structure (v3):
  - DMA batched into ~0.5MB triggers spread across sync/scalar/gpsimd
    queues in consumption order (~320 GB/s sustained).
  - q/k projections run h-pair-interleaved kc sweeps so the supply-paced
    first sweep keeps the PE above the HAM idle threshold.
  - Attention: S^T blocks for a (head, q-chunk) pass packed into a 2-bank
    PSUM tile (tail blocks share a bank column-wise), ONE batched exp per
    pass (ACTIVATE costs (N+352)/1.2ns so per-block exps were ACT-bound).
    Passes pipelined one ahead; att/den accumulators live in a paired
    2-bank tile double-buffered across heads so the next head's AV never
    waits on the previous head's normalization.
  - PSUM budget: sg[2 banks]x2 + ad[2 banks]x2 = exactly 8 banks.
  - Output partials written fp16 (halves the output-DMA tail).

All on-device layouts are feature-major so no transposes are needed:
  - x is shipped pre-transposed per batch: xt [2048, 1024] (fp16)
  - q, k are produced feature-major [dh, T] per head; v token-major [T, dh]
  - S^T[kv, q] = kf.T @ qf ; softmax denominator via ones[128,128] matmul
  - attention output accumulates as out^T[dh, q] = v_tm.T @ exp(S^T)
  - out^T is exactly the lhsT the out-projection needs
"""

import numpy as np

B, T, C = 4, 1024, 2048
H = 16          # total heads
HL = 8          # heads per core (local)
HB = 4          # heads per block
DH = 128        # head dim
KC = C // 128   # contraction chunks (16)
P = 128
NCORES = 8
BW = HB * DH    # head-block feature width (512)

_cache = {}


def _attn_passes(qc):
    """Pass layout for one (head, q-chunk): list of passes; each pass is a
    list of (bank, col_off, j, n, c0, diag) packed into one 2-bank PSUM
    slot. j = kv block index, n = #q columns computed, c0 = q-col offset
    within the 512-wide q window (n == 512 - c0), diag = causal mask on
    first 128 cols. Used prefix of the slot is contiguous -> one exp."""
    if qc == 0:
        return [
            [(0, 0, 0, 512, 0, True),
             (1, 0, 1, 384, 128, True),
             (1, 384, 3, 128, 384, True)],
            [(0, 0, 2, 256, 256, True)],
        ]
    else:
        return [
            [(0, 0, 0, 512, 0, False), (1, 0, 1, 512, 0, False)],
            [(0, 0, 2, 512, 0, False), (1, 0, 3, 512, 0, False)],
            [(0, 0, 4, 512, 0, True),
             (1, 0, 5, 384, 128, True),
             (1, 384, 7, 128, 384, True)],
            [(0, 0, 6, 256, 256, True)],
        ]


def _pass_used_cols(pss):
    return max(bank * 512 + off + n for bank, off, j, n, c0, diag in pss)


def _build():
    import concourse.bacc as bacc
    import concourse.mybir as mybir
    import concourse.tile as tile

    F32 = mybir.dt.float32
    F16 = mybir.dt.float16
    AF = mybir.ActivationFunctionType
    ALU = mybir.AluOpType

    nc = bacc.Bacc("TRN2", target_bir_lowering=False, debug=False)

    xt_d = nc.dram_tensor("xt", (C, T), F16, kind="ExternalInput")
    wq_d = nc.dram_tensor("wq", (C, HL * DH), F16, kind="ExternalInput")
    wk_d = nc.dram_tensor("wk", (C, HL * DH), F16, kind="ExternalInput")
    wv_d = nc.dram_tensor("wv", (C, HL * DH), F16, kind="ExternalInput")
    wo_d = nc.dram_tensor("wo", (HL * DH, C), F16, kind="ExternalInput")
    bq_d = nc.dram_tensor("bq", (P, HL), F32, kind="ExternalInput")
    bk_d = nc.dram_tensor("bk", (P, HL), F32, kind="ExternalInput")
    bvb_d = nc.dram_tensor("bvb", (P, HL * DH), F32, kind="ExternalInput")
    trip_d = nc.dram_tensor("trip", (P, 2 * 512), F16, kind="ExternalInput")
    tri1_d = nc.dram_tensor("tri1", (P, P), F16, kind="ExternalInput")
    part_d = nc.dram_tensor("part", (T, C), F16, kind="ExternalOutput")

    # grouped views for batched DMA
    xt_v = xt_d.rearrange("(g k p) t -> p g k t", p=P, k=2)       # 8 groups of 2
    wq_v = wq_d.rearrange("(g k p) m -> p g k m", p=P, k=4)       # 4 groups of 4
    wk_v = wk_d.rearrange("(g k p) m -> p g k m", p=P, k=4)
    wv_v = wv_d.rearrange("(g k p) m -> p g k m", p=P, k=4)
    wo_v = wo_d.rearrange("(h p) n -> p h n", p=P)
    part_v = part_d.rearrange("(mo p) n -> p mo n", p=P)

    with tile.TileContext(nc) as tc:
        with (
            tc.tile_pool(name="res", bufs=1) as res,
            tc.tile_pool(name="wblk", bufs=1) as wblk,
            tc.tile_pool(name="qkv", bufs=2) as qkv,
            tc.tile_pool(name="wp", bufs=3) as wp,
            tc.tile_pool(name="ps", bufs=1, space="PSUM") as ps,
        ):
            bq_sb = res.tile([P, HL], F32, tag="bq")
            bk_sb = res.tile([P, HL], F32, tag="bk")
            bvb_sb = res.tile([P, HL * DH], F32, tag="bvb")
            trip_sb = res.tile([P, 2 * 512], F16, tag="trip")
            tri1_sb = res.tile([P, P], F16, tag="tri1")

            ones_sb = res.tile([P, P], F16, tag="ones")
            nc.vector.memset(ones_sb[:], 1.0)

            # x^T in 16 single-kc chunks (smooth supply pacing)
            xts = [res.tile([P, T], F16, tag=f"xt{g}", name=f"xt{g}")
                   for g in range(KC)]

            def xt_ap(kc):
                return xts[kc][:]

            # weight chunks: 4 groups of 4 kc per name; tiles reused for blk1
            wts = {w: [wblk.tile([P, 4, BW], F16, tag=f"{w}{g}", name=f"{w}{g}")
                       for g in range(4)] for w in ("wq", "wk", "wv")}

            def w_ap(wname, kc):
                return wts[wname][kc // 4][:, kc % 4, :]

            wo_sb = res.tile([P, HL, C], F16, tag="wo")
            oT = res.tile([P, HL, T], F16, tag="oT")

            # ---- DMA: block 0 inputs, consumption-ordered, multi-queue ----
            for g in range(8):
                nc.sync.dma_start(xts[2 * g][:], xt_v[:, g, 0, :])
                nc.sync.dma_start(xts[2 * g + 1][:], xt_v[:, g, 1, :])
            for g in range(4):
                nc.scalar.dma_start(wts["wq"][g][:, 0:2, :], wq_v[:, g, 0:2, 0:BW])
                nc.scalar.dma_start(wts["wq"][g][:, 2:4, :], wq_v[:, g, 2:4, 0:BW])
            for g in range(4):
                nc.scalar.dma_start(wts["wk"][g][:], wk_v[:, g, :, 0:BW])
            nc.gpsimd.dma_start(bq_sb[:], bq_d[:])
            nc.gpsimd.dma_start(bk_sb[:], bk_d[:])
            nc.gpsimd.dma_start(bvb_sb[:], bvb_d[:])
            nc.gpsimd.dma_start(trip_sb[:], trip_d[:])
            nc.gpsimd.dma_start(tri1_sb[:], tri1_d[:])
            for g in range(4):
                nc.gpsimd.dma_start(wts["wv"][g][:], wv_v[:, g, :, 0:BW])

            # Warm the PE (HAM un-throttles after ~3.4us) until first data.
            warm = ps.tile([P, 2, 512], F32, tag="sg", bufs=2)
            for i in range(32):
                nc.tensor.matmul(
                    warm[:, i % 2, 0:P], ones_sb[:], ones_sb[:],
                    start=True, stop=True,
                )

            qf = {}
            kf = {}
            vt = {}

            def proj_qk(blk):
                """q/k projections for one head-block; h-pairs share a kc
                sweep so the supply-paced first sweep keeps the PE busy."""
                qf[blk] = qkv.tile([P, HB, T], F16, tag="qf", name=f"qf{blk}")
                kf[blk] = qkv.tile([P, HB, T], F16, tag="kf", name=f"kf{blk}")
                for dst, wname, bsb in (
                    (qf[blk], "wq", bq_sb), (kf[blk], "wk", bk_sb)
                ):
                    for hp in range(HB // 2):
                        slots = [
                            ps.tile([P, 2, 512], F32, tag="sg", bufs=2,
                                    name=f"p{wname}{blk}_{hp}_{i}")
                            for i in range(2)
                        ]
                        for kc in range(KC):
                            for i in range(2):
                                h = 2 * hp + i
                                w_sl = w_ap(wname, kc)[:, h * DH:(h + 1) * DH]
                                for t in range(2):
                                    nc.tensor.matmul(
                                        slots[i][:, t, :],
                                        w_sl,
                                        xt_ap(kc)[:, t * 512:(t + 1) * 512],
                                        start=(kc == 0),
                                        stop=(kc == KC - 1),
                                    )
                        for i in range(2):
                            h = 2 * hp + i
                            for t in range(2):
                                nc.vector.tensor_tensor(
                                    dst[:, h, t * 512:(t + 1) * 512],
                                    slots[i][:, t, :],
                                    bsb[
                                        :, blk * HB + h: blk * HB + h + 1
                                    ].to_broadcast((P, 512)),
                                    ALU.add,
                                )

            def proj_v(blk):
                lo = blk * BW
                vt[blk] = qkv.tile([P, T // P, BW], F16, tag="vt",
                                   name=f"vt{blk}")
                for m in range(T // P):
                    pt = ps.tile([P, 2, 512], F32, tag="sg", bufs=2)
                    for kc in range(KC):
                        nc.tensor.matmul(
                            pt[:, 0, :],
                            xt_ap(kc)[:, m * P:(m + 1) * P],
                            w_ap("wv", kc),
                            start=(kc == 0),
                            stop=(kc == KC - 1),
                        )
                    nc.vector.tensor_tensor(
                        vt[blk][:, m, :], pt[:, 0, :],
                        bvb_sb[:, lo:lo + BW], ALU.add,
                    )

            # ---------------- attention: pipelined passes ----------------
            def attn_block(blk):
                work = []
                for l in range(HB):
                    for qc in range(2):
                        pss = _attn_passes(qc)
                        for pi in range(len(pss)):
                            work.append((l, qc, pss, pi))

                state = {}

                def emit_S(l, qc, pss, pi):
                    sgrp = ps.tile(
                        [P, 2, 512], F32, tag="sg", bufs=2,
                        name=f"sg{blk}_{l}_{qc}_{pi}",
                    )
                    used = _pass_used_cols(pss[pi])
                    ndiag = sum(1 for e in pss[pi] if e[5])
                    for bank, off, j, n, c0, diag in pss[pi]:
                        nc.tensor.matmul(
                            sgrp[:, bank, off:off + n],
                            kf[blk][:, l, j * P:(j + 1) * P],
                            qf[blk][:, l, qc * 512 + c0: (qc + 1) * 512],
                            start=(off == 0),
                            stop=(off + n == 512)
                            or (bank * 512 + off + n == used),
                            skip_group_check=True,
                        )
                    E = wp.tile([P, 2, 512], F16, tag="E", bufs=4)
                    sflat = sgrp.rearrange("p a b -> p (a b)")
                    eflat = E.rearrange("p a b -> p (a b)")
                    nc.scalar.activation(eflat[:, :used], sflat[:, :used], AF.Exp)
                    # causal mask: zero the upper-tri wedges of diagonal
                    # blocks on the (otherwise idle) GpSimd engine, off the
                    # S->exp critical path.
                    if ndiag == 3:
                        nc.gpsimd.tensor_tensor(
                            eflat[:, :1024], eflat[:, :1024],
                            trip_sb[:], ALU.mult,
                        )
                    elif ndiag == 1:
                        nc.gpsimd.tensor_tensor(
                            eflat[:, :P], eflat[:, :P],
                            tri1_sb[:], ALU.mult,
                        )
                    return E

                def emit_AVden(l, qc, pss, pi, E):
                    st = state[(l, qc)]
                    first = pi == 0
                    last = pi == len(pss) - 1
                    np_ = len(pss[pi])
                    for idx, (bank, off, j, n, c0, diag) in enumerate(pss[pi]):
                        nc.tensor.matmul(
                            st["ad"][:, 0, c0:512],
                            vt[blk][:, j, l * DH:(l + 1) * DH],
                            E[:, bank, off:off + n],
                            start=(first and idx == 0),
                            stop=(last and idx == np_ - 1),
                            skip_group_check=True,
                        )
                    for idx, (bank, off, j, n, c0, diag) in enumerate(pss[pi]):
                        nc.tensor.matmul(
                            st["ad"][:, 1, c0:512],
                            ones_sb[:],
                            E[:, bank, off:off + n],
                            start=(first and idx == 0),
                            stop=(last and idx == np_ - 1),
                            skip_group_check=True,
                        )
                    if last:
                        hh = blk * HB + l
                        rc = wp.tile([P, 512], F32, tag="rc")
                        nc.vector.reciprocal_approx_fast(rc[:], st["ad"][:, 1, :])
                        nc.vector.tensor_tensor(
                            oT[:, hh, qc * 512:(qc + 1) * 512],
                            st["ad"][:, 0, :], rc[:], ALU.mult,
                        )

                pending = None
                for (l, qc, pss, pi) in work:
                    if pi == 0:
                        state[(l, qc)] = {
                            "ad": ps.tile([P, 2, 512], F32, tag="ad", bufs=2,
                                          name=f"ad{blk}_{l}_{qc}"),
                        }
                    E = emit_S(l, qc, pss, pi)
                    if pending is not None:
                        emit_AVden(*pending)
                    pending = (l, qc, pss, pi, E)
                if pending is not None:
                    emit_AVden(*pending)

            # ---------------- block 0 ----------------
            proj_qk(0)
            proj_v(0)

            # block-1 weights (WAR on block-0 consumption); wo late
            for g in range(4):
                nc.scalar.dma_start(wts["wq"][g][:], wq_v[:, g, :, BW:2 * BW])
            for g in range(4):
                nc.scalar.dma_start(wts["wk"][g][:], wk_v[:, g, :, BW:2 * BW])
            for g in range(4):
                nc.gpsimd.dma_start(wts["wv"][g][:], wv_v[:, g, :, BW:2 * BW])
            nc.gpsimd.dma_start(wo_sb[:], wo_v[:, :, :])

            attn_block(0)

            # ---------------- block 1 ----------------
            proj_qk(1)
            proj_v(1)
            attn_block(1)

            # ---------------- out projection ----------------
            for m in range(T // P):
                po = wp.tile([P, 4, 512], F16, tag="po", bufs=2)
                for ng in range(2):
                    pt = ps.tile([P, 2, 512], F32, tag="sg", bufs=2,
                                 name=f"op{m}_{ng}")
                    for h in range(HL):
                        for bi in range(2):
                            n2 = 2 * ng + bi
                            nc.tensor.matmul(
                                pt[:, bi, :],
                                oT[:, h, m * P:(m + 1) * P],
                                wo_sb[:, h, n2 * 512:(n2 + 1) * 512],
                                start=(h == 0),
                                stop=(h == HL - 1),
                            )
                    for bi in range(2):
                        nc.vector.tensor_copy(po[:, 2 * ng + bi, :], pt[:, bi, :])
                nc.sync.dma_start(part_v[:, m, :], po.rearrange("p a b -> p (a b)"))

    nc.compile()
    return nc


def _prep_inputs(x, w_qkv, b_qkv, w_out):
    """Build the 8 per-core input maps (host-side shard + layout prep)."""
    f16 = np.float16
    scale = np.float32(1.0 / np.sqrt(DH))

    xt = [np.ascontiguousarray(x[b].T).astype(f16) for b in range(B)]

    # 0/1 upper-tri wedge masks applied to exp(S^T) blocks (kv on partitions,
    # q on free): zero where q_rel < kv_rel.
    tri1 = (np.arange(P)[None, :] >= np.arange(P)[:, None]).astype(f16)
    trip = np.ones((P, 1024), f16)
    for base in (0, 512, 896):
        trip[:, base:base + P] = tri1

    per_g = []
    for g in range(2):
        lo, hi = g * HL * DH, (g + 1) * HL * DH
        wq = np.ascontiguousarray(w_qkv[:, lo:hi] * scale).astype(f16)
        wk = np.ascontiguousarray(w_qkv[:, C + lo: C + hi]).astype(f16)
        wv = np.ascontiguousarray(w_qkv[:, 2 * C + lo: 2 * C + hi]).astype(f16)
        wo = np.ascontiguousarray(w_out[lo:hi, :]).astype(f16)
        bq = (b_qkv[lo:hi] * scale).astype(np.float32).reshape(HL, P).T.copy()
        bk = b_qkv[C + lo: C + hi].astype(np.float32).reshape(HL, P).T.copy()
        bv = b_qkv[2 * C + lo: 2 * C + hi].astype(np.float32)
        bvb = np.ascontiguousarray(np.broadcast_to(bv[None, :], (P, HL * DH)))
        per_g.append(dict(wq=wq, wk=wk, wv=wv, wo=wo, bq=bq, bk=bk, bvb=bvb))

    in_maps = []
    for c in range(NCORES):
        b, g = c // 2, c % 2
        m = dict(per_g[g])
        m["xt"] = xt[b]
        m["trip"] = trip
        m["tri1"] = tri1
        in_maps.append(m)
    return in_maps


def run(x, w_qkv, b_qkv, w_out, b_out, trace=False, **trace_kwargs):
    from concourse.bass_utils import run_bass_kernel_spmd

    x = np.asarray(x, dtype=np.float32)
    w_qkv = np.asarray(w_qkv, dtype=np.float32)
    b_qkv = np.asarray(b_qkv, dtype=np.float32)
    w_out = np.asarray(w_out, dtype=np.float32)
    b_out = np.asarray(b_out, dtype=np.float32)

    if "nc" not in _cache:
        _cache["nc"] = _build()
    nc = _cache["nc"]

    in_maps = _prep_inputs(x, w_qkv, b_qkv, w_out)
    res = run_bass_kernel_spmd(
        nc, in_maps, core_ids=list(range(NCORES)), trace=trace, **trace_kwargs
    )

    out = np.empty((B, T, C), np.float32)
    for b in range(B):
        out[b] = (res.results[2 * b]["part"].astype(np.float32)
                  + res.results[2 * b + 1]["part"].astype(np.float32))
    out += b_out
    return out, res


def kernel(x, w_qkv, b_qkv, w_out, b_out):
    out, _ = run(x, w_qkv, b_qkv, w_out, b_out)
    return out


# revision 8
# speedup vs baseline: 1.2429x; 1.2429x over previous
"""Multi-head causal self-attention (B=4, T=1024, d_model=2048, 16 heads of 128)
for 8 Trainium2 NeuronCores.

Sharding: hybrid data x tensor parallel. Core c handles batch b = c//2 and
head group g = c%2 (8 heads per core). Each core computes q/k/v projections
for its 8 heads, causal flash-style attention, and the out-projection rows
for those heads, producing a partial [1024, 2048] output for its batch.
The host sums the two partials per batch and adds the output bias.

All on-device layouts are feature-major so no transposes are needed anywhere:
  - x is shipped pre-transposed per batch: xt [2048, 1024] (fp16)
  - q, k are produced feature-major [dh, T] per head; v token-major [T, dh]
  - scores are computed transposed: S^T[kv, q] = k_fm.T @ q_fm (lhsT=k, rhs=q)
  - softmax denominator via ones[128,128] matmul (partition reduction on PE),
    which also broadcasts the per-q sum to all 128 partitions
  - attention output accumulates as out^T[dh, q] = v_tm.T @ exp(S^T)
  - out^T is exactly the lhsT the out-projection needs

Heads are processed in two blocks of 4 so projection weights and q/k/v
activations fit in SBUF alongside the resident x^T and w_out. Within a
block, attention is computed for two heads interleaved so PE matmuls hide
the ACT exp latency. Inputs are DMA'd in per-k-chunk tiles so the first
projection matmuls start ~2us in instead of waiting for monolithic loads.
"""

import numpy as np

B, T, C = 4, 1024, 2048
H = 16          # total heads
HL = 8          # heads per core (local)
HB = 4          # heads per block
DH = 128        # head dim
KC = C // 128   # contraction chunks (16)
P = 128
NCORES = 8

_cache = {}


def _build():
    import concourse.bacc as bacc
    import concourse.mybir as mybir
    import concourse.tile as tile

    F32 = mybir.dt.float32
    F16 = mybir.dt.float16
    AF = mybir.ActivationFunctionType
    ALU = mybir.AluOpType

    nc = bacc.Bacc("TRN2", target_bir_lowering=False, debug=False)

    xt_d = nc.dram_tensor("xt", (C, T), F16, kind="ExternalInput")
    wq_d = nc.dram_tensor("wq", (C, HL * DH), F16, kind="ExternalInput")
    wk_d = nc.dram_tensor("wk", (C, HL * DH), F16, kind="ExternalInput")
    wv_d = nc.dram_tensor("wv", (C, HL * DH), F16, kind="ExternalInput")
    wo_d = nc.dram_tensor("wo", (HL * DH, C), F16, kind="ExternalInput")
    bq_d = nc.dram_tensor("bq", (P, HL), F32, kind="ExternalInput")
    bk_d = nc.dram_tensor("bk", (P, HL), F32, kind="ExternalInput")
    bvb_d = nc.dram_tensor("bvb", (P, HL * DH), F32, kind="ExternalInput")
    mask_d = nc.dram_tensor("mask", (P, P), F32, kind="ExternalInput")
    part_d = nc.dram_tensor("part", (T, C), F16, kind="ExternalOutput")

    BW = HB * DH  # head-block feature width (512)

    xt_v = xt_d.rearrange("(o p) t -> p o t", p=P)
    wq_v = wq_d.rearrange("(g k p) m -> p g k m", p=P, k=4)
    wk_v = wk_d.rearrange("(g k p) m -> p g k m", p=P, k=4)
    wv_v = wv_d.rearrange("(g k p) m -> p g k m", p=P, k=4)
    wo_v = wo_d.rearrange("(h p) n -> p h n", p=P)
    part_v = part_d.rearrange("(mo p) n -> p mo n", p=P)

    with tile.TileContext(nc) as tc:
        with (
            tc.tile_pool(name="res", bufs=1) as res,
            tc.tile_pool(name="wblk", bufs=1) as wblk,
            tc.tile_pool(name="qkv", bufs=2) as qkv,
            tc.tile_pool(name="wp", bufs=3) as wp,
            tc.tile_pool(name="ps", bufs=3, space="PSUM") as ps,
        ):
            bq_sb = res.tile([P, HL], F32, tag="bq")
            bk_sb = res.tile([P, HL], F32, tag="bk")
            bvb_sb = res.tile([P, HL * DH], F32, tag="bvb")
            mask_sb = res.tile([P, P], F32, tag="mask")

            ones_sb = res.tile([P, P], F16, tag="ones")
            nc.vector.memset(ones_sb[:], 1.0)

            # Warm the PE (HAM un-throttles after ~3.4us of activity) while the
            # input DMAs stream in; these matmuls depend only on the memset.
            warm = ps.tile([P, P], F32, tag="mm")
            for _ in range(48):
                nc.tensor.matmul(warm[:], ones_sb[:], ones_sb[:], start=True, stop=True)

            # x^T in per-k-chunk tiles so compute starts after the first chunks
            xts = [res.tile([P, T], F16, tag=f"xt{kc}", name=f"xt{kc}")
                   for kc in range(KC)]
            wgrp = {w: [wblk.tile([P, 4, BW], F16, tag=f"{w}{g}", name=f"{w}{g}")
                        for g in range(4)] for w in ("wq", "wk", "wv")}

            def w_ap(wname, kc):
                return wgrp[wname][kc // 4][:, kc % 4, :]

            def dma_block_weights(blk):
                lo = blk * BW
                if blk == 0:
                    # multi-queue, consumption-ordered: x on sync; wq then wk
                    # on scalar; biases/mask then wv on gpsimd.
                    for kc in range(KC):
                        nc.sync.dma_start(xts[kc][:], xt_v[:, kc, :])
                    for g in range(4):
                        nc.scalar.dma_start(
                            wgrp["wq"][g][:, 0:2, :], wq_v[:, g, 0:2, lo:lo + BW])
                        nc.scalar.dma_start(
                            wgrp["wq"][g][:, 2:4, :], wq_v[:, g, 2:4, lo:lo + BW])
                    for g in range(4):
                        nc.scalar.dma_start(
                            wgrp["wk"][g][:], wk_v[:, g, :, lo:lo + BW])
                    nc.gpsimd.dma_start(bq_sb[:], bq_d[:])
                    nc.gpsimd.dma_start(bk_sb[:], bk_d[:])
                    nc.gpsimd.dma_start(bvb_sb[:], bvb_d[:])
                    nc.gpsimd.dma_start(mask_sb[:], mask_d[:])
                    for g in range(4):
                        nc.gpsimd.dma_start(
                            wgrp["wv"][g][:], wv_v[:, g, :, lo:lo + BW])
                else:
                    for g in range(4):
                        nc.scalar.dma_start(
                            wgrp["wq"][g][:], wq_v[:, g, :, lo:lo + BW])
                    for g in range(4):
                        nc.scalar.dma_start(
                            wgrp["wk"][g][:], wk_v[:, g, :, lo:lo + BW])
                    for g in range(4):
                        nc.gpsimd.dma_start(
                            wgrp["wv"][g][:], wv_v[:, g, :, lo:lo + BW])
                    nc.gpsimd.dma_start(
                        wo_sb[:], wo_v[:, :, :])

            wo_sb = res.tile([P, HL, C], F16, tag="wo")
            oT = res.tile([P, HL, T], F16, tag="oT")

            for blk in range(HL // HB):
                lo = blk * BW
                dma_block_weights(blk)

                qf = qkv.tile([P, HB, T], F16, tag="qf")
                kf = qkv.tile([P, HB, T], F16, tag="kf")
                vt = qkv.tile([P, T // P, BW], F16, tag="vt")

                # ---- Phase 1: projections for this block ----
                # dst-outer so wk isn't demanded until the full wq+x sweep
                # is done (k-proj starts ~40us in; wk arrives ~30us).
                for dst, wname, bsb in (("qf", "wq", bq_sb), ("kf", "wk", bk_sb)):
                    dtile = qf if dst == "qf" else kf
                    for h in range(HB):
                        for t in range(T // 512):
                            pt = ps.tile([P, 512], F32, tag="mm")
                            for kc in range(KC):
                                nc.tensor.matmul(
                                    pt[:],
                                    w_ap(wname, kc)[:, h * DH : (h + 1) * DH],
                                    xts[kc][:, t * 512 : (t + 1) * 512],
                                    start=(kc == 0),
                                    stop=(kc == KC - 1),
                                )
                            nc.vector.tensor_tensor(
                                dtile[:, h, t * 512 : (t + 1) * 512],
                                pt[:],
                                bsb[
                                    :, blk * HB + h : blk * HB + h + 1
                                ].to_broadcast((P, 512)),
                                ALU.add,
                            )
                for m in range(T // P):
                    pt = ps.tile([P, 512], F32, tag="mm")
                    for kc in range(KC):
                        nc.tensor.matmul(
                            pt[:],
                            xts[kc][:, m * P : (m + 1) * P],
                            w_ap("wv", kc),
                            start=(kc == 0),
                            stop=(kc == KC - 1),
                        )
                    nc.vector.tensor_tensor(
                        vt[:, m, :], pt[:], bvb_sb[:, lo : lo + BW], ALU.add
                    )

                # ---- Phase 2: causal attention, two heads interleaved ----
                for hp in range(HB // 2):
                    pair = (2 * hp, 2 * hp + 1)  # local head idx within block
                    for qc in range(T // 512):
                        jmax = (qc + 1) * 4
                        att = {}
                        den = {}
                        for l in pair:
                            att[l] = ps.tile(
                                [P, 512], F32, tag="att", bufs=3, name=f"att{l}"
                            )
                            den[l] = ps.tile(
                                [P, 512], F32, tag="den", bufs=2, name=f"den{l}"
                            )

                        def bounds(j):
                            s = max(512 * qc, 128 * j)
                            return s, 512 * qc + 512 - s

                        sts = {}

                        def issue_st(l, j):
                            s, n = bounds(j)
                            st = ps.tile([P, 512], F32, tag="mm", name=f"st{l}")
                            nc.tensor.matmul(
                                st[:, :n],
                                kf[:, l, j * P : (j + 1) * P],
                                qf[:, l, s : 512 * qc + 512],
                                start=True,
                                stop=True,
                            )
                            if 128 * j >= 512 * qc:
                                nc.vector.tensor_tensor(
                                    st[:, :P], st[:, :P], mask_sb[:], ALU.add
                                )
                            sts[(l, j)] = st

                        for l in pair:
                            issue_st(l, 0)
                        for j in range(jmax):
                            s, n = bounds(j)
                            c0 = s - 512 * qc
                            for l in pair:
                                st = sts.pop((l, j))
                                E = wp.tile([P, 512], F16, tag="E", bufs=6)
                                nc.scalar.activation(E[:, :n], st[:, :n], AF.Exp)
                                if j + 1 < jmax:
                                    issue_st(l, j + 1)
                                nc.tensor.matmul(
                                    att[l][:, c0:],
                                    vt[:, j, l * DH : (l + 1) * DH],
                                    E[:, :n],
                                    start=(j == 0),
                                    stop=(j == jmax - 1),
                                )
                                nc.tensor.matmul(
                                    den[l][:, c0:],
                                    ones_sb[:],
                                    E[:, :n],
                                    start=(j == 0),
                                    stop=(j == jmax - 1),
                                )
                        for l in pair:
                            hh = blk * HB + l
                            rc = wp.tile([P, 512], F32, tag="rc")
                            nc.vector.reciprocal_approx_fast(rc[:], den[l][:])
                            nc.vector.tensor_tensor(
                                oT[:, hh, qc * 512 : (qc + 1) * 512],
                                att[l][:],
                                rc[:],
                                ALU.mult,
                            )

            # ---- Phase 3: out projection (partial over this core's heads) ----
            for m in range(T // P):
                po = wp.tile([P, 4, 512], F16, tag="po", bufs=2)
                for n2 in range(C // 512):
                    pt = ps.tile([P, 512], F32, tag="mm")
                    for h in range(HL):
                        nc.tensor.matmul(
                            pt[:],
                            oT[:, h, m * P : (m + 1) * P],
                            wo_sb[:, h, n2 * 512 : (n2 + 1) * 512],
                            start=(h == 0),
                            stop=(h == HL - 1),
                        )
                    if n2 % 2 == 0:
                        nc.vector.tensor_copy(po[:, n2, :], pt[:])
                    else:
                        nc.scalar.activation(
                            po[:, n2, :], pt[:], AF.Copy)
                nc.sync.dma_start(
                    part_v[:, m, :], po.rearrange("p a b -> p (a b)"))

    nc.compile()
    return nc


def _prep_inputs(x, w_qkv, b_qkv, w_out):
    """Build the 8 per-core input maps (host-side shard + layout prep)."""
    f16 = np.float16
    scale = np.float32(1.0 / np.sqrt(DH))

    xt = [np.ascontiguousarray(x[b].T).astype(f16) for b in range(B)]

    mask = np.where(
        np.arange(P)[None, :] >= np.arange(P)[:, None], 0.0, -1e30
    ).astype(np.float32)

    per_g = []
    for g in range(2):
        lo, hi = g * HL * DH, (g + 1) * HL * DH
        wq = np.ascontiguousarray(w_qkv[:, lo:hi] * scale).astype(f16)
        wk = np.ascontiguousarray(w_qkv[:, C + lo : C + hi]).astype(f16)
        wv = np.ascontiguousarray(w_qkv[:, 2 * C + lo : 2 * C + hi]).astype(f16)
        wo = np.ascontiguousarray(w_out[lo:hi, :]).astype(f16)
        bq = (b_qkv[lo:hi] * scale).astype(np.float32).reshape(HL, P).T.copy()
        bk = b_qkv[C + lo : C + hi].astype(np.float32).reshape(HL, P).T.copy()
        bv = b_qkv[2 * C + lo : 2 * C + hi].astype(np.float32)
        bvb = np.ascontiguousarray(np.broadcast_to(bv[None, :], (P, HL * DH)))
        per_g.append(dict(wq=wq, wk=wk, wv=wv, wo=wo, bq=bq, bk=bk, bvb=bvb))

    in_maps = []
    for c in range(NCORES):
        b, g = c // 2, c % 2
        m = dict(per_g[g])
        m["xt"] = xt[b]
        m["mask"] = mask
        in_maps.append(m)
    return in_maps


def run(x, w_qkv, b_qkv, w_out, b_out, trace=False, **trace_kwargs):
    from concourse.bass_utils import run_bass_kernel_spmd

    x = np.asarray(x, dtype=np.float32)
    w_qkv = np.asarray(w_qkv, dtype=np.float32)
    b_qkv = np.asarray(b_qkv, dtype=np.float32)
    w_out = np.asarray(w_out, dtype=np.float32)
    b_out = np.asarray(b_out, dtype=np.float32)

    if "nc" not in _cache:
        _cache["nc"] = _build()
    nc = _cache["nc"]

    in_maps = _prep_inputs(x, w_qkv, b_qkv, w_out)
    res = run_bass_kernel_spmd(
        nc, in_maps, core_ids=list(range(NCORES)), trace=trace, **trace_kwargs
    )

    out = np.empty((B, T, C), np.float32)
    for b in range(B):
        out[b] = (res.results[2 * b]["part"].astype(np.float32)
                  + res.results[2 * b + 1]["part"].astype(np.float32))
    out += b_out
    return out, res


def kernel(x, w_qkv, b_qkv, w_out, b_out):
    out, _ = run(x, w_qkv, b_qkv, w_out, b_out)
    return out

